# revision 21
# baseline (speedup 1.0000x reference)
"""Bass/Tile kernel for bidirectional multi-head self-attention on 8 trn2 cores.

Problem: x[4, 2048, 1024], W_qkv[3072, 1024], W_proj[1024, 1024], H=16 heads,
Dh=64.  out = proj(softmax(q k^T / sqrt(Dh)) v).

Sharding: core c = (batch b = c//2, head-group g = c%2).  Each core computes
attention for 8 heads of one batch and a full-T partial output projection
(contraction over its 512 C_in columns); host sums the pair partials and
stacks batches.  x is transposed on the host (xT [D,T]) so no on-chip
transposes are needed; the output is produced as out^T [D,T] and transposed
back on the host.

Per-core device pipeline (matmuls bf16/fp16 in, fp32 psum):
  phase 1: qT/kT [dh,T] via W-stationary matmuls over xT; v [T,dh] via
           xT-stationary matmuls; v packed per head pair as
           [v_even | ones | v_odd] so each head's AV lhsT [128] slice embeds
           a ones block: AV psum rows carry y on one half and the softmax
           denominator replicated on the other half.
  phase 2: per head-pair: score matmuls for both heads issued adjacently on
           disjoint PE row groups (rows 0-63 / 64-127) -> concurrent on the
           128x128 array.  exp split across engines: ScalarE true exp (fp16
           out) for most tiles, VectorE Schraudolph exp (fp32->int16
           tensor_scalar, bits reinterpreted as fp16) for the rest.  AV
           accumulates over kt into ps_y; normalization = reciprocal of the
           replicated denominator rows + partition-shift DMA + DVE multiply
           straight into persistent yT tiles.
  phase 3: outT[o,t] = sum_hp wp[hp]^T yT[hp] with W stationary, DMA to DRAM.
"""

import os
import numpy as np
import ml_dtypes

import concourse.bass as bass
import concourse.bacc as bacc
import concourse.mybir as mybir
import concourse.tile as tile
from concourse.bass_utils import run_bass_kernel_spmd

# ---- problem constants (hardcoded per harness contract) --------------------
B = 4
T = 2048
D = 1024
H = 16
DH = 64
N_CORES = 8
HPC = H // 2          # heads per core = 8
F = HPC * DH          # 512 = per-core q/k/v feature width
NPAIR = HPC // 2      # 4 head pairs per core

NCC = D // 128        # 8 contraction chunks over D
NKT = T // 128        # 16 key tiles
NQC = T // 512        # 4 query chunks

F32 = mybir.dt.float32
F16 = mybir.dt.float16
I16 = mybir.dt.int16
BF16 = mybir.dt.bfloat16

DT = BF16
NP_DT = ml_dtypes.bfloat16

# Schraudolph fp16 exp: bits = round(s * ALPHA + BETA), viewed as fp16,
# approximates exp(s/8).  (1024 mantissa-steps per octave, 15 = fp16 bias.)
EXP_ALPHA = 1024.0 / (8.0 * float(np.log(2.0)))
EXP_BETA = 15.0 * 1024.0 - 44.0

# kts whose exp runs as VectorE Schraudolph (the rest use ScalarE true exp).
# Alternating keeps the per-kt exp latency off a single engine's queue; kt 15
# goes to ScalarE since its latency is exposed in the drain tail anyway.
DVE_KT = frozenset({1, 3, 5, 7, 9, 11, 13})

LAST_EXEC_NS = None
LAST_RESULTS = None


def build_program(debug=False):
    nc = bacc.Bacc()

    xT_d = nc.dram_tensor("x_t", [D, T], DT, kind="ExternalInput")
    wqkv_d = nc.dram_tensor("w_qkv_t", [D, 3 * F], DT, kind="ExternalInput")
    wproj_d = nc.dram_tensor("w_proj_t", [F, D], DT, kind="ExternalInput")
    out_d = nc.dram_tensor("out_p", [D, T], BF16, kind="ExternalOutput")
    dbg = {}
    if debug:
        dbg["qkT0"] = nc.dram_tensor("dbg_qkT0", [128, T], DT,
                                     kind="ExternalOutput")
        dbg["qkT4"] = nc.dram_tensor("dbg_qkT4", [128, T], DT,
                                     kind="ExternalOutput")
        dbg["vaug0"] = nc.dram_tensor("dbg_vaug0", [128, NPAIR * 192], F16,
                                      kind="ExternalOutput")
        dbg["attA0"] = nc.dram_tensor("dbg_attA0", [128, 512], F16,
                                      kind="ExternalOutput")
        dbg["attB0"] = nc.dram_tensor("dbg_attB0", [128, 512], F16,
                                      kind="ExternalOutput")
        dbg["psyA0"] = nc.dram_tensor("dbg_psyA0", [128, 512], F32,
                                      kind="ExternalOutput")
        dbg["psyB0"] = nc.dram_tensor("dbg_psyB0", [128, 512], F32,
                                      kind="ExternalOutput")
        dbg["yT0"] = nc.dram_tensor("dbg_yT0", [128, T], DT,
                                    kind="ExternalOutput")

    with tile.TileContext(nc) as tc:
        with (
            tc.tile_pool(name="consts", bufs=1) as consts,
            tc.tile_pool(name="qk_pool", bufs=1) as qk_pool,
            tc.tile_pool(name="v_pool", bufs=1) as v_pool,
            tc.tile_pool(name="y_pool", bufs=1) as y_pool,
            tc.tile_pool(name="wp_pool", bufs=1) as wp_pool,
        ):
            # persistent tensors
            # qkT[f]: f 0..3 -> qT for head pair f (rows 0:64 head 2f, rows
            # 64:128 head 2f+1); f 4..7 -> kT for pair f-4.
            qkT = [qk_pool.tile([128, T], DT, name=f"qkT{f}") for f in range(8)]
            # v_aug[tt]: [128 t, 4 pairs * 192]; per pair [v_even|ones|v_odd]
            v_aug = [v_pool.tile([128, NPAIR * 192], F16, name=f"vaug{t}")
                     for t in range(NKT)]
            # yT[hp]: [128 dh (2 heads), T]
            yT = [y_pool.tile([128, T], DT, name=f"yT{hp}") for hp in range(4)]
            # W_proj^T slice tiles [128 dh, D]
            wp_sb = [wp_pool.tile([128, D], DT, name=f"wp{i}") for i in range(4)]

            # warm the ScalarE exp table during initial DMA wait
            warm_in = consts.tile([128, 16], F32)
            warm_out = consts.tile([128, 16], F16)
            nc.gpsimd.memset(warm_in, 0.0)
            nc.scalar.activation(warm_out, warm_in,
                                 mybir.ActivationFunctionType.Exp)

            # ones blocks of v_aug never change: memset once (gpsimd)
            for tt in range(NKT):
                va_r = v_aug[tt].rearrange("p (pr x d) -> p pr x d",
                                           pr=NPAIR, x=3)
                nc.gpsimd.memset(va_r[:, :, 1, :], 1.0)

            # ---------------- phase 1: qkv projection ----------------------
            with (
                tc.tile_pool(name="ph1_w", bufs=1) as ph1_w,
                tc.tile_pool(name="ph1_x", bufs=1) as ph1_x,
                tc.tile_pool(name="ph1_psum", bufs=1, space="PSUM") as ph1_p,
            ):
                xT_sb = [ph1_x.tile([128, T], DT, name=f"xT{cc}")
                         for cc in range(NCC)]
                w_sb = [ph1_w.tile([128, 3 * F], DT, name=f"wqkv{cc}")
                        for cc in range(NCC)]
                # first tiles split into chunks so the first qk chain (which
                # needs only x0's first t-chunk and w0's q columns) starts as
                # early as possible
                nc.sync.dma_start(out=xT_sb[0][:, 0:512],
                                  in_=xT_d[0:128, 0:512])
                nc.sync.dma_start(out=w_sb[0][:, 0:512],
                                  in_=wqkv_d[0:128, 0:512])
                nc.sync.dma_start(out=xT_sb[0][:, 512:T],
                                  in_=xT_d[0:128, 512:T])
                nc.sync.dma_start(out=w_sb[0][:, 512:3 * F],
                                  in_=wqkv_d[0:128, 512:3 * F])
                for cc in range(1, NCC):
                    nc.sync.dma_start(out=xT_sb[cc],
                                      in_=xT_d[cc * 128:(cc + 1) * 128, :])
                    nc.sync.dma_start(out=w_sb[cc],
                                      in_=wqkv_d[cc * 128:(cc + 1) * 128, :])
                for i in range(4):
                    nc.sync.dma_start(out=wp_sb[i],
                                      in_=wproj_d[i * 128:(i + 1) * 128, :])

                # q/k: out [f 128, t 512] = w_slice^T @ xT
                for f in range(8):
                    wcol = f * 128 if f < 4 else F + (f - 4) * 128
                    for tq in range(4):
                        ps_qk = ph1_p.tile([128, 512], F32, name="ps_qk",
                                           tag="ps_qk", bufs=4)
                        for cc in range(NCC):
                            nc.tensor.matmul(
                                ps_qk,
                                lhsT=w_sb[cc][:, wcol:wcol + 128],
                                rhs=xT_sb[cc][:, tq * 512:(tq + 1) * 512],
                                start=(cc == 0), stop=(cc == NCC - 1))
                        dst = qkT[f][:, tq * 512:(tq + 1) * 512]
                        if tq % 2 == 0:
                            nc.scalar.activation(
                                dst, ps_qk, mybir.ActivationFunctionType.Copy)
                        else:
                            nc.vector.tensor_copy(dst, ps_qk)

                # v: out [t 128, f 512] = xT_slice^T @ w_v
                for tt in range(NKT):
                    ps_v = ph1_p.tile([128, F], F32, name="ps_v",
                                      tag="ps_v", bufs=4)
                    for cc in range(NCC):
                        nc.tensor.matmul(
                            ps_v,
                            lhsT=xT_sb[cc][:, tt * 128:(tt + 1) * 128],
                            rhs=w_sb[cc][:, 2 * F:3 * F],
                            start=(cc == 0), stop=(cc == NCC - 1))
                    va_r = v_aug[tt].rearrange("p (pr x d) -> p pr x d",
                                               pr=NPAIR, x=3)
                    ps_r = ps_v.rearrange("p (pr x d) -> p pr x d",
                                          pr=NPAIR, x=2)
                    nc.vector.tensor_copy(va_r[:, :, 0, :], ps_r[:, :, 0, :])
                    nc.vector.tensor_copy(va_r[:, :, 2, :], ps_r[:, :, 1, :])

            # ---------------- phase 2: attention ---------------------------
            with (
                tc.tile_pool(name="ph2_s", bufs=1) as ph2_s,
                tc.tile_pool(name="ph2_d", bufs=2, space="DRAM") as ph2_d,
                tc.tile_pool(name="ph2_psum", bufs=1, space="PSUM") as ph2_p,
            ):
                for pair in range(NPAIR):
                    qT = qkT[pair]
                    kT = qkT[4 + pair]
                    vcolA = pair * 192          # [v_even | ones]
                    vcolB = pair * 192 + 64     # [ones | v_odd]
                    for qc in range(NQC):
                        q0 = qc * 512
                        ps_yA = ph2_p.tile([128, 512], F32, name="ps_yA",
                                           tag="ps_yA", bufs=2)
                        ps_yB = ph2_p.tile([128, 512], F32, name="ps_yB",
                                           tag="ps_yB", bufs=2)
                        attq = []
                        for blk in range(NKT // 2 + 1):
                            # scores + exp for the two kts of this block,
                            # then the AVs of the previous block (depth-2
                            # software pipeline).  One psum tile holds both
                            # heads' scores so a single exp op produces them
                            # together: the following A/B score MMs become
                            # ready at the same instant and issue adjacently
                            # (disjoint PE row groups -> concurrent).
                            for kt in (2 * blk, 2 * blk + 1):
                                if kt >= NKT:
                                    continue
                                ps_ab = ph2_p.tile([128, 1024], F32,
                                                   name="ps_ab", tag="ps_ab",
                                                   bufs=2)
                                k0 = kt * 128
                                nc.tensor.matmul(
                                    ps_ab[:, 0:512],
                                    lhsT=kT[0:64, k0:k0 + 128],
                                    rhs=qT[0:64, q0:q0 + 512],
                                    start=True, stop=True)
                                nc.tensor.matmul(
                                    ps_ab[:, 512:1024],
                                    lhsT=kT[64:128, k0:k0 + 128],
                                    rhs=qT[64:128, q0:q0 + 512],
                                    start=True, stop=True)
                                if kt not in DVE_KT:
                                    expf = ph2_s.tile([128, 1024], F16,
                                                      name="expf",
                                                      tag="expf", bufs=3)
                                    nc.scalar.activation(
                                        expf, ps_ab,
                                        mybir.ActivationFunctionType.Exp,
                                        scale=1.0 / 8.0)
                                    att = expf
                                else:
                                    bits = ph2_s.tile([128, 1024], I16,
                                                      name="bits",
                                                      tag="bits", bufs=3)
                                    nc.vector.tensor_scalar(
                                        bits, ps_ab, EXP_ALPHA, EXP_BETA,
                                        mybir.AluOpType.mult,
                                        mybir.AluOpType.add)
                                    att = bits.bitcast(F16)
                                if debug and pair == 0 and qc == 0 and kt == 0:
                                    nc.sync.dma_start(out=dbg["attA0"][:, :],
                                                      in_=att[:, 0:512])
                                    nc.sync.dma_start(out=dbg["attB0"][:, :],
                                                      in_=att[:, 512:1024])
                                attq.append(att)
                            for ka in (2 * blk - 2, 2 * blk - 1):
                                if ka < 0:
                                    continue
                                a_ap = attq[ka]
                                nc.tensor.matmul(
                                    ps_yA,
                                    lhsT=v_aug[ka][:, vcolA:vcolA + 128],
                                    rhs=a_ap[:, 0:512],
                                    start=(ka == 0), stop=(ka == NKT - 1))
                                nc.tensor.matmul(
                                    ps_yB,
                                    lhsT=v_aug[ka][:, vcolB:vcolB + 128],
                                    rhs=a_ap[:, 512:1024],
                                    start=(ka == 0), stop=(ka == NKT - 1))
                        if debug and pair == 0 and qc == 0:
                            psyA_sb = ph2_s.tile([128, 512], F32,
                                                 name="psyA_sb", bufs=1)
                            nc.vector.tensor_copy(psyA_sb, ps_yA)
                            nc.sync.dma_start(out=dbg["psyA0"][:, :], in_=psyA_sb)
                            psyB_sb = ph2_s.tile([128, 512], F32,
                                                 name="psyB_sb", bufs=1)
                            nc.vector.tensor_copy(psyB_sb, ps_yB)
                            nc.sync.dma_start(out=dbg["psyB0"][:, :], in_=psyB_sb)
                        # normalization. A: y rows 0:64, den rows 64:128.
                        # B: den rows 0:64, y rows 64:128.
                        # head A: den lives at partitions 64:128 but the mul
                        # must run at partitions 0:64.  SBUF->SBUF DMA with a
                        # base-64 source reads wrong data on HW, so bounce
                        # through DRAM (proven pattern).
                        dA = ph2_s.tile([128, 512], F32, name="dA",
                                        tag="dA", bufs=2)
                        nc.vector.tensor_copy(dA[64:128, :], ps_yA[64:128, :])
                        d_dr = ph2_d.tile([64, 512], F32, name="d_dr",
                                          tag="d_dr")
                        nc.sync.dma_start(out=d_dr, in_=dA[64:128, :])
                        dAs = ph2_s.tile([128, 512], F32, name="dAs",
                                         tag="dAs", bufs=2)
                        nc.sync.dma_start(out=dAs[0:64, :], in_=d_dr)
                        rA = ph2_s.tile([128, 512], F32, name="rA",
                                        tag="rA", bufs=2)
                        nc.vector.reciprocal_approx_fast(
                            rA[0:64, :], dAs[0:64, :])
                        nc.vector.tensor_mul(
                            yT[pair][0:64, q0:q0 + 512],
                            ps_yA[0:64, :], rA[0:64, :])
                        rB = ph2_s.tile([128, 512], F32, name="rB",
                                        tag="rB", bufs=2)
                        rBs = ph2_s.tile([128, 512], F32, name="rBs",
                                         tag="rBs", bufs=2)
                        nc.vector.reciprocal_approx_fast(
                            rB[0:64, :], ps_yB[0:64, :])
                        nc.sync.dma_start(out=rBs[64:128, :], in_=rB[0:64, :])
                        nc.vector.tensor_mul(
                            yT[pair][64:128, q0:q0 + 512],
                            ps_yB[64:128, :], rBs[64:128, :])

            if debug:
                nc.sync.dma_start(out=dbg["qkT0"][:, :], in_=qkT[0])
                nc.sync.dma_start(out=dbg["qkT4"][:, :], in_=qkT[4])
                nc.sync.dma_start(out=dbg["vaug0"][:, :], in_=v_aug[0])
                nc.sync.dma_start(out=dbg["yT0"][:, :], in_=yT[0])

            # ---------------- phase 3: output projection -------------------
            with (
                tc.tile_pool(name="ph3_s", bufs=1) as ph3_s,
                tc.tile_pool(name="ph3_psum", bufs=1, space="PSUM") as ph3_p,
            ):
                for oc in range(8):
                    for tcix in range(4):
                        ps_o = ph3_p.tile([128, 512], F32, name="ps_o",
                                          tag="ps_o", bufs=8)
                        for hp in range(4):
                            nc.tensor.matmul(
                                ps_o,
                                lhsT=wp_sb[hp][:, oc * 128:(oc + 1) * 128],
                                rhs=yT[hp][:, tcix * 512:(tcix + 1) * 512],
                                start=(hp == 0), stop=(hp == 3))
                        o_sb = ph3_s.tile([128, 512], BF16, name="o_sb",
                                          tag="o_sb", bufs=4)
                        if tcix % 2 == 0:
                            nc.vector.tensor_copy(o_sb, ps_o)
                        else:
                            nc.scalar.activation(
                                o_sb, ps_o, mybir.ActivationFunctionType.Copy)
                        nc.sync.dma_start(
                            out=out_d[oc * 128:(oc + 1) * 128,
                                      tcix * 512:(tcix + 1) * 512],
                            in_=o_sb)
    return nc


_NC_CACHE = None


def _get_program():
    global _NC_CACHE
    if _NC_CACHE is None:
        nc = build_program()
        if not nc.is_finalized():
            nc.finalize()
        _NC_CACHE = nc
    return _NC_CACHE


def make_in_maps(x, W_qkv, W_proj):
    """Shard full inputs into per-core input maps (host-side layout prep)."""
    Wq, Wk, Wv = W_qkv[0:D], W_qkv[D:2 * D], W_qkv[2 * D:3 * D]
    maps = []
    wq_g, wp_g = {}, {}
    for g in range(2):
        rows = slice(g * F, (g + 1) * F)
        wq_g[g] = np.ascontiguousarray(
            np.concatenate([Wq[rows].T, Wk[rows].T, Wv[rows].T], axis=1)
        ).astype(NP_DT)
        wp_g[g] = np.ascontiguousarray(W_proj[:, rows].T).astype(NP_DT)
    xt_b = {}
    for b in range(B):
        xt_b[b] = np.ascontiguousarray(x[b].T).astype(NP_DT)
    for core in range(N_CORES):
        b, g = core // 2, core % 2
        maps.append({
            "x_t": xt_b[b],
            "w_qkv_t": wq_g[g],
            "w_proj_t": wp_g[g],
        })
    return maps


def kernel(x, W_qkv, W_proj):
    global LAST_EXEC_NS, LAST_RESULTS
    x = np.asarray(x, dtype=np.float32)
    W_qkv = np.asarray(W_qkv, dtype=np.float32)
    W_proj = np.asarray(W_proj, dtype=np.float32)

    nc = _get_program()
    in_maps = make_in_maps(x, W_qkv, W_proj)
    trace = bool(int(os.environ.get("BASS_KERNEL_TRACE", "0")))
    res = run_bass_kernel_spmd(nc, in_maps, list(range(N_CORES)), trace=trace)
    LAST_EXEC_NS = res.exec_time_ns
    LAST_RESULTS = res
    out = np.stack([
        (np.asarray(res.results[2 * b]["out_p"], dtype=np.float32)
         + np.asarray(res.results[2 * b + 1]["out_p"], dtype=np.float32)).T
        for b in range(B)
    ])
    return np.ascontiguousarray(out)


# revision 22
# speedup vs baseline: 1.1737x; 1.1737x over previous
"""Bass/Tile kernel for bidirectional multi-head self-attention on 8 trn2 cores.

Problem: x[4, 2048, 1024], W_qkv[3072, 1024], W_proj[1024, 1024], H=16 heads,
Dh=64.  out = proj(softmax(q k^T / sqrt(Dh)) v).

Sharding: core c = (batch b = c//2, head-group g = c%2).  Each core computes
attention for 8 heads of one batch and a full-T partial output projection
(contraction over its 512 C_in columns); host sums the pair partials and
stacks batches.  x is transposed on the host (xT [D,T]) so no on-chip
transposes are needed; the output is produced as out^T [D,T] and transposed
back on the host.

Per-core device pipeline (matmuls bf16/fp16 in, fp32 psum):
  phase 1: qT/kT [dh,T] via W-stationary matmuls over xT; v [T,dh] via
           xT-stationary matmuls; v packed per head pair as
           [v_even | ones | v_odd] so each head's AV lhsT [128] slice embeds
           a ones block: AV psum rows carry y on one half and the softmax
           denominator replicated on the other half.
  phase 2: per head-pair: score matmuls for both heads issued adjacently on
           disjoint PE row groups (rows 0-63 / 64-127) -> concurrent on the
           128x128 array.  exp split across engines: ScalarE true exp (fp16
           out) for most tiles, VectorE Schraudolph exp (fp32->int16
           tensor_scalar, bits reinterpreted as fp16) for the rest.  AV
           accumulates over kt into ps_y; normalization = reciprocal of the
           replicated denominator rows + partition-shift DMA + DVE multiply
           straight into persistent yT tiles.
  phase 3: outT[o,t] = sum_hp wp[hp]^T yT[hp] with W stationary, DMA to DRAM.
"""

import os
import numpy as np
import ml_dtypes

import concourse.bass as bass
import concourse.bacc as bacc
import concourse.mybir as mybir
import concourse.tile as tile
from concourse.bass_utils import run_bass_kernel_spmd

# ---- problem constants (hardcoded per harness contract) --------------------
B = 4
T = 2048
D = 1024
H = 16
DH = 64
N_CORES = 8
HPC = H // 2          # heads per core = 8
F = HPC * DH          # 512 = per-core q/k/v feature width
NPAIR = HPC // 2      # 4 head pairs per core

NCC = D // 128        # 8 contraction chunks over D
NKT = T // 128        # 16 key tiles
NQC = T // 512        # 4 query chunks

F32 = mybir.dt.float32
F16 = mybir.dt.float16
I16 = mybir.dt.int16
BF16 = mybir.dt.bfloat16

DT = BF16
NP_DT = ml_dtypes.bfloat16

# Schraudolph fp16 exp: bits = round(s * ALPHA + BETA), viewed as fp16,
# approximates exp(s/8).  (1024 mantissa-steps per octave, 15 = fp16 bias.)
EXP_ALPHA = 1024.0 / (8.0 * float(np.log(2.0)))
EXP_BETA = 15.0 * 1024.0 - 44.0

# kts whose exp runs as VectorE Schraudolph (the rest use ScalarE true exp).
# Alternating keeps the per-kt exp latency off a single engine's queue; kt 15
# goes to ScalarE since its latency is exposed in the drain tail anyway.
DVE_KT = frozenset({1, 3, 5, 7, 9, 11, 13})

LAST_EXEC_NS = None
LAST_RESULTS = None


def build_program(debug=False):
    nc = bacc.Bacc()

    xT_d = nc.dram_tensor("x_t", [D, T], DT, kind="ExternalInput")
    wqkv_d = nc.dram_tensor("w_qkv_t", [D, 3 * F], DT, kind="ExternalInput")
    wproj_d = nc.dram_tensor("w_proj_t", [F, D], DT, kind="ExternalInput")
    out_d = nc.dram_tensor("out_p", [D, T], BF16, kind="ExternalOutput")
    dbg = {}
    if debug:
        dbg["qkT0"] = nc.dram_tensor("dbg_qkT0", [128, T], DT,
                                     kind="ExternalOutput")
        dbg["qkT4"] = nc.dram_tensor("dbg_qkT4", [128, T], DT,
                                     kind="ExternalOutput")
        dbg["vaug0"] = nc.dram_tensor("dbg_vaug0", [128, NPAIR * 192], F16,
                                      kind="ExternalOutput")
        dbg["attA0"] = nc.dram_tensor("dbg_attA0", [128, 512], F16,
                                      kind="ExternalOutput")
        dbg["attB0"] = nc.dram_tensor("dbg_attB0", [128, 512], F16,
                                      kind="ExternalOutput")
        dbg["psyA0"] = nc.dram_tensor("dbg_psyA0", [128, 512], F32,
                                      kind="ExternalOutput")
        dbg["psyB0"] = nc.dram_tensor("dbg_psyB0", [128, 512], F32,
                                      kind="ExternalOutput")
        dbg["yT0"] = nc.dram_tensor("dbg_yT0", [128, T], DT,
                                    kind="ExternalOutput")

    with tile.TileContext(nc) as tc:
        with (
            tc.tile_pool(name="consts", bufs=1) as consts,
            tc.tile_pool(name="qk_pool", bufs=1) as qk_pool,
            tc.tile_pool(name="v_pool", bufs=1) as v_pool,
            tc.tile_pool(name="y_pool", bufs=1) as y_pool,
            tc.tile_pool(name="wp_pool", bufs=1) as wp_pool,
        ):
            # persistent tensors
            # qkT[f]: f 0..3 -> qT for head pair f (rows 0:64 head 2f, rows
            # 64:128 head 2f+1); f 4..7 -> kT for pair f-4.
            qkT = [qk_pool.tile([128, T], DT, name=f"qkT{f}") for f in range(8)]
            # v_aug[tt]: [128 t, 4 pairs * 192]; per pair [v_even|ones|v_odd]
            v_aug = [v_pool.tile([128, NPAIR * 192], F16, name=f"vaug{t}")
                     for t in range(NKT)]
            # yT[hp]: [128 dh (2 heads), T]
            yT = [y_pool.tile([128, T], DT, name=f"yT{hp}") for hp in range(4)]
            # W_proj^T slice tiles [128 dh, D]
            wp_sb = [wp_pool.tile([128, D], DT, name=f"wp{i}") for i in range(4)]

            # warm the ScalarE exp table during initial DMA wait
            warm_in = consts.tile([128, 16], F32)
            warm_out = consts.tile([128, 16], F16)
            nc.gpsimd.memset(warm_in, 0.0)
            nc.scalar.activation(warm_out, warm_in,
                                 mybir.ActivationFunctionType.Exp)

            # ones blocks of v_aug never change: memset once (gpsimd)
            for tt in range(NKT):
                va_r = v_aug[tt].rearrange("p (pr x d) -> p pr x d",
                                           pr=NPAIR, x=3)
                nc.gpsimd.memset(va_r[:, :, 1, :], 1.0)

            # ---------------- phase 1: qkv projection ----------------------
            with (
                tc.tile_pool(name="ph1_w", bufs=1) as ph1_w,
                tc.tile_pool(name="ph1_x", bufs=1) as ph1_x,
                tc.tile_pool(name="ph1_psum", bufs=1, space="PSUM") as ph1_p,
            ):
                xT_sb = [ph1_x.tile([128, T], DT, name=f"xT{cc}")
                         for cc in range(NCC)]
                w_sb = [ph1_w.tile([128, 3 * F], DT, name=f"wqkv{cc}")
                        for cc in range(NCC)]
                # first tiles split into chunks so the first qk chain (which
                # needs only x0's first t-chunk and w0's q columns) starts as
                # early as possible
                nc.sync.dma_start(out=xT_sb[0][:, 0:512],
                                  in_=xT_d[0:128, 0:512])
                nc.sync.dma_start(out=w_sb[0][:, 0:512],
                                  in_=wqkv_d[0:128, 0:512])
                nc.sync.dma_start(out=xT_sb[0][:, 512:T],
                                  in_=xT_d[0:128, 512:T])
                nc.sync.dma_start(out=w_sb[0][:, 512:3 * F],
                                  in_=wqkv_d[0:128, 512:3 * F])
                for cc in range(1, NCC):
                    nc.sync.dma_start(out=xT_sb[cc],
                                      in_=xT_d[cc * 128:(cc + 1) * 128, :])
                    nc.sync.dma_start(out=w_sb[cc],
                                      in_=wqkv_d[cc * 128:(cc + 1) * 128, :])
                for i in range(4):
                    nc.sync.dma_start(out=wp_sb[i],
                                      in_=wproj_d[i * 128:(i + 1) * 128, :])

                # q/k: out [f 128, t 512] = w_slice^T @ xT
                for f in range(8):
                    wcol = f * 128 if f < 4 else F + (f - 4) * 128
                    for tq in range(4):
                        ps_qk = ph1_p.tile([128, 512], F32, name="ps_qk",
                                           tag="ps_qk", bufs=4)
                        for cc in range(NCC):
                            nc.tensor.matmul(
                                ps_qk,
                                lhsT=w_sb[cc][:, wcol:wcol + 128],
                                rhs=xT_sb[cc][:, tq * 512:(tq + 1) * 512],
                                start=(cc == 0), stop=(cc == NCC - 1))
                        dst = qkT[f][:, tq * 512:(tq + 1) * 512]
                        if tq % 2 == 0:
                            nc.scalar.activation(
                                dst, ps_qk, mybir.ActivationFunctionType.Copy)
                        else:
                            nc.vector.tensor_copy(dst, ps_qk)

                # v: out [t 128, f 512] = xT_slice^T @ w_v
                for tt in range(NKT):
                    ps_v = ph1_p.tile([128, F], F32, name="ps_v",
                                      tag="ps_v", bufs=4)
                    for cc in range(NCC):
                        nc.tensor.matmul(
                            ps_v,
                            lhsT=xT_sb[cc][:, tt * 128:(tt + 1) * 128],
                            rhs=w_sb[cc][:, 2 * F:3 * F],
                            start=(cc == 0), stop=(cc == NCC - 1))
                    va_r = v_aug[tt].rearrange("p (pr x d) -> p pr x d",
                                               pr=NPAIR, x=3)
                    ps_r = ps_v.rearrange("p (pr x d) -> p pr x d",
                                          pr=NPAIR, x=2)
                    nc.vector.tensor_copy(va_r[:, :, 0, :], ps_r[:, :, 0, :])
                    nc.vector.tensor_copy(va_r[:, :, 2, :], ps_r[:, :, 1, :])

            # ---------------- phase 2: attention ---------------------------
            with (
                tc.tile_pool(name="ph2_s", bufs=1) as ph2_s,
                tc.tile_pool(name="ph2_d", bufs=2, space="DRAM") as ph2_d,
                tc.tile_pool(name="ph2_psum", bufs=1, space="PSUM") as ph2_p,
            ):
                for pair in range(NPAIR):
                    qT = qkT[pair]
                    kT = qkT[4 + pair]
                    vcolA = pair * 192          # [v_even | ones]
                    vcolB = pair * 192 + 64     # [ones | v_odd]
                    for qc in range(NQC):
                        q0 = qc * 512
                        ps_yA = ph2_p.tile([128, 512], F32, name="ps_yA",
                                           tag="ps_yA", bufs=2)
                        ps_yB = ph2_p.tile([128, 512], F32, name="ps_yB",
                                           tag="ps_yB", bufs=2)
                        attq = []
                        for kt in range(NKT + 2):
                            if kt < NKT:
                                # one psum tile holds both heads' scores so a
                                # single exp op produces them together: the
                                # next kt's A/B score MMs become ready at the
                                # same instant and issue adjacently (disjoint
                                # PE row groups -> concurrent).
                                ps_ab = ph2_p.tile([128, 1024], F32,
                                                   name="ps_ab", tag="ps_ab",
                                                   bufs=2)
                                k0 = kt * 128
                                nc.tensor.matmul(
                                    ps_ab[:, 0:512],
                                    lhsT=kT[0:64, k0:k0 + 128],
                                    rhs=qT[0:64, q0:q0 + 512],
                                    start=True, stop=True)
                                nc.tensor.matmul(
                                    ps_ab[:, 512:1024],
                                    lhsT=kT[64:128, k0:k0 + 128],
                                    rhs=qT[64:128, q0:q0 + 512],
                                    start=True, stop=True)
                                if kt % 2 == 0:
                                    expf = ph2_s.tile([128, 1024], F16,
                                                      name="expf",
                                                      tag="expf", bufs=3)
                                    nc.scalar.activation(
                                        expf, ps_ab,
                                        mybir.ActivationFunctionType.Exp,
                                        scale=1.0 / 8.0)
                                    att = expf
                                else:
                                    bits = ph2_s.tile([128, 1024], I16,
                                                      name="bits",
                                                      tag="bits", bufs=3)
                                    nc.vector.tensor_scalar(
                                        bits, ps_ab, EXP_ALPHA, EXP_BETA,
                                        mybir.AluOpType.mult,
                                        mybir.AluOpType.add)
                                    att = bits.bitcast(F16)
                                if debug and pair == 0 and qc == 0 and kt == 0:
                                    nc.sync.dma_start(out=dbg["attA0"][:, :],
                                                      in_=att[:, 0:512])
                                    nc.sync.dma_start(out=dbg["attB0"][:, :],
                                                      in_=att[:, 512:1024])
                                attq.append(att)
                            if kt >= 2:
                                ka = kt - 2
                                a_ap = attq[ka]
                                nc.tensor.matmul(
                                    ps_yA,
                                    lhsT=v_aug[ka][:, vcolA:vcolA + 128],
                                    rhs=a_ap[:, 0:512],
                                    start=(ka == 0), stop=(ka == NKT - 1))
                                nc.tensor.matmul(
                                    ps_yB,
                                    lhsT=v_aug[ka][:, vcolB:vcolB + 128],
                                    rhs=a_ap[:, 512:1024],
                                    start=(ka == 0), stop=(ka == NKT - 1))
                        if debug and pair == 0 and qc == 0:
                            psyA_sb = ph2_s.tile([128, 512], F32,
                                                 name="psyA_sb", bufs=1)
                            nc.vector.tensor_copy(psyA_sb, ps_yA)
                            nc.sync.dma_start(out=dbg["psyA0"][:, :], in_=psyA_sb)
                            psyB_sb = ph2_s.tile([128, 512], F32,
                                                 name="psyB_sb", bufs=1)
                            nc.vector.tensor_copy(psyB_sb, ps_yB)
                            nc.sync.dma_start(out=dbg["psyB0"][:, :], in_=psyB_sb)
                        # normalization. A: y rows 0:64, den rows 64:128.
                        # B: den rows 0:64, y rows 64:128.
                        # head A: den lives at partitions 64:128 but the mul
                        # must run at partitions 0:64.  SBUF->SBUF DMA with a
                        # base-64 source reads wrong data on HW, so bounce
                        # through DRAM (proven pattern).
                        dA = ph2_s.tile([128, 512], F32, name="dA",
                                        tag="dA", bufs=2)
                        nc.vector.tensor_copy(dA[64:128, :], ps_yA[64:128, :])
                        d_dr = ph2_d.tile([64, 512], F32, name="d_dr",
                                          tag="d_dr")
                        nc.sync.dma_start(out=d_dr, in_=dA[64:128, :])
                        dAs = ph2_s.tile([128, 512], F32, name="dAs",
                                         tag="dAs", bufs=2)
                        nc.sync.dma_start(out=dAs[0:64, :], in_=d_dr)
                        rA = ph2_s.tile([128, 512], F32, name="rA",
                                        tag="rA", bufs=2)
                        nc.vector.reciprocal_approx_fast(
                            rA[0:64, :], dAs[0:64, :])
                        nc.vector.tensor_mul(
                            yT[pair][0:64, q0:q0 + 512],
                            ps_yA[0:64, :], rA[0:64, :])
                        rB = ph2_s.tile([128, 512], F32, name="rB",
                                        tag="rB", bufs=2)
                        rBs = ph2_s.tile([128, 512], F32, name="rBs",
                                         tag="rBs", bufs=2)
                        nc.vector.reciprocal_approx_fast(
                            rB[0:64, :], ps_yB[0:64, :])
                        nc.sync.dma_start(out=rBs[64:128, :], in_=rB[0:64, :])
                        nc.vector.tensor_mul(
                            yT[pair][64:128, q0:q0 + 512],
                            ps_yB[64:128, :], rBs[64:128, :])

            if debug:
                nc.sync.dma_start(out=dbg["qkT0"][:, :], in_=qkT[0])
                nc.sync.dma_start(out=dbg["qkT4"][:, :], in_=qkT[4])
                nc.sync.dma_start(out=dbg["vaug0"][:, :], in_=v_aug[0])
                nc.sync.dma_start(out=dbg["yT0"][:, :], in_=yT[0])

            # ---------------- phase 3: output projection -------------------
            with (
                tc.tile_pool(name="ph3_s", bufs=1) as ph3_s,
                tc.tile_pool(name="ph3_psum", bufs=1, space="PSUM") as ph3_p,
            ):
                for oc in range(8):
                    for tcix in range(4):
                        ps_o = ph3_p.tile([128, 512], F32, name="ps_o",
                                          tag="ps_o", bufs=8)
                        for hp in range(4):
                            nc.tensor.matmul(
                                ps_o,
                                lhsT=wp_sb[hp][:, oc * 128:(oc + 1) * 128],
                                rhs=yT[hp][:, tcix * 512:(tcix + 1) * 512],
                                start=(hp == 0), stop=(hp == 3))
                        o_sb = ph3_s.tile([128, 512], BF16, name="o_sb",
                                          tag="o_sb", bufs=4)
                        if tcix % 2 == 0:
                            nc.vector.tensor_copy(o_sb, ps_o)
                        else:
                            nc.scalar.activation(
                                o_sb, ps_o, mybir.ActivationFunctionType.Copy)
                        nc.sync.dma_start(
                            out=out_d[oc * 128:(oc + 1) * 128,
                                      tcix * 512:(tcix + 1) * 512],
                            in_=o_sb)
    return nc


_NC_CACHE = None


def _get_program():
    global _NC_CACHE
    if _NC_CACHE is None:
        nc = build_program()
        if not nc.is_finalized():
            nc.finalize()
        _NC_CACHE = nc
    return _NC_CACHE


def make_in_maps(x, W_qkv, W_proj):
    """Shard full inputs into per-core input maps (host-side layout prep)."""
    Wq, Wk, Wv = W_qkv[0:D], W_qkv[D:2 * D], W_qkv[2 * D:3 * D]
    maps = []
    wq_g, wp_g = {}, {}
    for g in range(2):
        rows = slice(g * F, (g + 1) * F)
        wq_g[g] = np.ascontiguousarray(
            np.concatenate([Wq[rows].T, Wk[rows].T, Wv[rows].T], axis=1)
        ).astype(NP_DT)
        wp_g[g] = np.ascontiguousarray(W_proj[:, rows].T).astype(NP_DT)
    xt_b = {}
    for b in range(B):
        xt_b[b] = np.ascontiguousarray(x[b].T).astype(NP_DT)
    for core in range(N_CORES):
        b, g = core // 2, core % 2
        maps.append({
            "x_t": xt_b[b],
            "w_qkv_t": wq_g[g],
            "w_proj_t": wp_g[g],
        })
    return maps


def kernel(x, W_qkv, W_proj):
    global LAST_EXEC_NS, LAST_RESULTS
    x = np.asarray(x, dtype=np.float32)
    W_qkv = np.asarray(W_qkv, dtype=np.float32)
    W_proj = np.asarray(W_proj, dtype=np.float32)

    nc = _get_program()
    in_maps = make_in_maps(x, W_qkv, W_proj)
    trace = bool(int(os.environ.get("BASS_KERNEL_TRACE", "0")))
    res = run_bass_kernel_spmd(nc, in_maps, list(range(N_CORES)), trace=trace)
    LAST_EXEC_NS = res.exec_time_ns
    LAST_RESULTS = res
    out = np.stack([
        (np.asarray(res.results[2 * b]["out_p"], dtype=np.float32)
         + np.asarray(res.results[2 * b + 1]["out_p"], dtype=np.float32)).T
        for b in range(B)
    ])
    return np.ascontiguousarray(out)
